# revision 1
# baseline (speedup 1.0000x reference)
"""Bilinear causal self-attention kernel for Trainium2, 8-core SPMD.

Problem: h [4,2048,512] f32, A [8,512,512] f32.
  scores_h = (h @ A_h) @ h^T   (per head, causal mask, softmax)
  out = attn @ h, heads concatenated -> [4, 2048, 4096]

Sharding: one head per NeuronCore (8 heads, 8 cores). Each core gets the
full h plus its own A_h; outputs are concatenated on the feature dim.

Per-core algorithm (all in "transposed score" [s, t] layout; softmax needs
no max-subtraction because |scores| < ~75 so exp stays in fp32 range):
  hT  [d, t]  = PE-transpose of h
  hAT [e, t]  = A^T chunks @ hT      (e on partitions)
  S^T [s, t]  = hT chunks^T @ hAT    (accumulate over e)
  E = exp(S^T + causal_mask)
  denom[1, t] = ones^T @ (E_j + E_j+1)  (PE matmul per chunk pair)
  out[t, d]   = sum_s E-chunks^T @ h[s, d]   (accumulate over s-chunks)
  out /= denom (per-partition scale after row->col matmul transpose)
Big matmuls run in float32r (full-rate rounded fp32, 1-8-11); operands are
rounded by the DVE/ACT copies that produce them (walrus requires this).
"""
import numpy as np

B, T, D, H = 4, 2048, 512, 8
NCORES = 8
TCH = T // 128          # 16 s-chunks (128 tokens) per batch
NTB = T // 512          # 4 t-blocks (512 tokens) per batch
NEG = -1.0e30

_cache = {}


def _build_nc(reps=1):
    from concourse import bacc
    import concourse.mybir as mybir
    from concourse.tile import TileContext
    from concourse.masks import make_identity
    f32 = mybir.dt.float32
    f32r = mybir.dt.float32r

    nc = bacc.Bacc("TRN2", target_bir_lowering=False, debug=False)
    h_d = nc.declare_dram_parameter("h", [B * T, D], f32, isOutput=False)
    A_d = nc.declare_dram_parameter("A", [D, D], f32, isOutput=False)
    o_d = nc.declare_dram_parameter("out", [B * T, D], f32, isOutput=True)

    with TileContext(nc) as tc:
        with tc.tile_pool(name="const", bufs=1) as const, \
             tc.tile_pool(name="cpool", bufs=4) as cpool, \
             tc.tile_pool(name="hpool", bufs=2) as hpool, \
             tc.tile_pool(name="tpool", bufs=1) as tpool, \
             tc.tile_pool(name="epool", bufs=3) as epool, \
             tc.tile_pool(name="opool", bufs=4) as opool, \
             tc.tile_pool(name="spool", bufs=2) as spool, \
             tc.tile_pool(name="ps", bufs=1, space="PSUM") as ps:

            ident = const.tile([128, 128], f32)
            make_identity(nc, ident)
            ident_r = const.tile([128, 128], f32r)
            nc.vector.tensor_copy(ident_r, ident)
            ones_f = const.tile([128, 1], f32)
            nc.vector.memset(ones_f, 1.0)
            ones_r = const.tile([128, 1], f32r)
            nc.vector.tensor_copy(ones_r, ones_f)
            # causal mask buffer [128, 896]: valid (0.0) iff c >= ss + 384,
            # else -1e30.  Slice [384-off : 896-off] gives the [128, 512]
            # additive mask for diagonal offset off = (j-4k)*128.
            mask = const.tile([128, 896], f32)
            nc.gpsimd.memset(mask, 0.0)
            nc.gpsimd.affine_select(
                out=mask, in_=mask,
                compare_op=mybir.AluOpType.is_ge,
                fill=NEG, base=-384,
                pattern=[[1, 896]], channel_multiplier=-1,
            )
            # A in [d, e] layout, rounded to f32r: [128 d, dc, 512 e]
            A_r = const.tile([128, 4, D], f32r)
            for dc in range(4):
                a_ch = cpool.tile([128, D], f32, tag="chunk", name=f"a_ch{dc}", bufs=6)
                nc.sync.dma_start(
                    out=a_ch,
                    in_=A_d[:, :][dc * 128:(dc + 1) * 128, :])
                nc.gpsimd.tensor_copy(A_r[:, dc, :], a_ch)

            for b in [bb for _ in range(reps) for bb in range(B)]:
                # ---- load h[b] chunks; transpose to hT; round to h_r ----
                h_r = hpool.tile([128, TCH, D], f32r, tag="h_r")
                hT = tpool.tile([128, 4, T], f32r, tag="hT")
                for jj in reversed(range(4)):
                    chunks = []
                    for r in range(4):
                        j = jj * 4 + r
                        hch = cpool.tile([128, D], f32, tag="chunk",
                                         name=f"hch{j}", bufs=6)
                        nc.sync.dma_start(
                            out=hch,
                            in_=h_d[:, :][b * T + j * 128:
                                          b * T + (j + 1) * 128, :])
                        nc.gpsimd.tensor_copy(h_r[:, j, :], hch)
                        chunks.append(hch)
                    for half in range(2):
                        # first group borrows the sc-tag banks: at a batch
                        # boundary the av slots are still held by the last
                        # attention blocks, the sc slots are already free
                        tg, bf = ("sc", 3) if jj == 3 else ("av", 4)
                        tps2 = [ps.tile([128, 512], f32, tag=tg, bufs=bf,
                                        name=f"tp{half}{i}") for i in range(2)]
                        for r in range(4):
                            for i in range(2):
                                # transpose the f32r-rounded copy (1.5 vs 2
                                # cyc/row); hT holds rounded values either way
                                dc = half * 2 + i
                                j = jj * 4 + r
                                nc.tensor.transpose(
                                    tps2[i][:, r * 128:(r + 1) * 128]
                                    .bitcast(f32r),
                                    h_r[:, j, dc * 128:(dc + 1) * 128],
                                    ident_r)
                        nc.scalar.copy(
                            hT[:, half * 2, jj * 512:(jj + 1) * 512], tps2[0])
                        nc.vector.tensor_copy(
                            hT[:, half * 2 + 1, jj * 512:(jj + 1) * 512],
                            tps2[1])
                # ---- build hAT [128 e, ec, T] ----
                hAT = tpool.tile([128, 4, T], f32r, tag="hAT")
                for k in reversed(range(NTB)):
                    for ec in range(4):
                        mm_ps = ps.tile([128, 512], f32, tag="sc", bufs=3)
                        for dc in range(4):
                            nc.tensor.matmul(
                                mm_ps,
                                A_r[:, dc, ec * 128:(ec + 1) * 128],
                                hT[:, dc, k * 512:(k + 1) * 512],
                                start=(dc == 0), stop=(dc == 3))
                        eng = nc.scalar.copy if ec < 2 else \
                            nc.vector.tensor_copy
                        eng(hAT[:, ec, k * 512:(k + 1) * 512], mm_ps)
                # ---- attention per t-block (emission software-pipelined:
                #      dn/av matmuls for chunk j-1 go after scores of j).
                # Descending k: frees high-j regions of hT/hAT/h_r early so
                # the next batch's build overlaps this batch's attention. ----
                pending = []

                def flush(keep):
                    while len(pending) > keep:
                        pending.pop(0)()

                for k in reversed(range(NTB)):
                    av = [ps.tile([128, D], f32, tag="av", bufs=4,
                                  name=f"av{m}") for m in range(4)]
                    dn = ps.tile([1, 512], f32, tag="dn", bufs=1)
                    nj = 4 * k + 4          # s-chunks 0..4k+3
                    Es = {}
                    E2s = {}

                    def consume(j, k=k, nj=nj, av=av, dn=dn, Es=Es, E2s=E2s):
                        E = Es[j]
                        # denominator: one PE matmul per QUAD of chunks; the
                        # pair/quad sums run on DVE (off the critical chain).
                        if j % 2 == 1:
                            E2 = epool.tile([128, 512], f32r, tag="E2",
                                            bufs=3)
                            nc.gpsimd.tensor_add(E2, Es[j - 1], E)
                            if j % 4 == 1:
                                E2s[0] = E2
                            else:
                                E4 = epool.tile([128, 512], f32r, tag="E4",
                                                bufs=2)
                                nc.gpsimd.tensor_add(E4, E2s.pop(0), E2)
                                nc.tensor.matmul(
                                    dn, ones_r[:, 0:1], E4,
                                    start=(j == 3), stop=(j == nj - 1))
                            del Es[j - 1]
                        for m in range(4):
                            lastj = 4 * k + m
                            if j <= lastj:
                                nc.tensor.matmul(
                                    av[m],
                                    E[:, m * 128:(m + 1) * 128],
                                    h_r[:, j, :],
                                    start=(j == 0), stop=(j == lastj))
                        if j % 2 == 1 or j == nj - 1:
                            Es.pop(j, None)

                    def tail(k=k, av=av, dn=dn, b=b):
                        # denom -> column, scale, store
                        dnrow = spool.tile([1, 512], f32, tag="dnrow")
                        nc.vector.tensor_copy(dnrow, dn)
                        dc_ps = ps.tile([128, 4], f32, tag="dn", bufs=1,
                                        name="dc_ps")
                        for m in range(4):
                            nc.tensor.matmul(
                                dc_ps[:, m:m + 1],
                                dnrow[0:1, m * 128:(m + 1) * 128],
                                ident[0:1, 0:1],
                                start=True, stop=True)
                        rdcol = spool.tile([128, 4], f32, tag="rdcol")
                        nc.vector.reciprocal(rdcol, dc_ps)
                        for m in range(4):
                            o_sb = opool.tile([128, D], f32, tag="o")
                            nc.vector.tensor_scalar_mul(
                                o_sb, av[m], rdcol[:, m:m + 1])
                            row0 = b * T + k * 512 + m * 128
                            nc.sync.dma_start(
                                out=o_d[:, :][row0: row0 + 128, :], in_=o_sb)

                    for j in range(nj):
                        S = ps.tile([128, 512], f32, tag="sc", bufs=3)
                        # Diagonal chunks: columns tt < off are fully masked;
                        # skip them in the matmul (c0 capped at 256 to keep
                        # the f32r moving dim >= 256) and memset them to -1e30
                        # (emitted first so DVE runs it off the chain).
                        off = (j - 4 * k) * 128 if j >= 4 * k else -1
                        c0 = min(off, 256) if off > 0 else 0
                        if c0 > 0:
                            nc.vector.memset(S[:, 0:c0], NEG)
                        for ec in range(4):
                            nc.tensor.matmul(
                                S[:, c0:512],
                                hT[:, ec, j * 128:(j + 1) * 128],
                                hAT[:, ec, k * 512 + c0:(k + 1) * 512],
                                start=(ec == 0), stop=(ec == 3))
                        if off >= 0:        # partial-diagonal region mask
                            lo = 384 - off + c0
                            nc.vector.tensor_add(
                                S[:, c0:off + 128], S[:, c0:off + 128],
                                mask[:, lo:512])
                        E = epool.tile([128, 512], f32r, tag="E", bufs=8)
                        nc.scalar.activation(
                            E, S, mybir.ActivationFunctionType.Exp)
                        Es[j] = E
                        pending.append(lambda j=j, c=consume: c(j))
                        flush(4)
                    pending.append(tail)
                flush(0)
    nc.compile()
    return nc


def _get_nc():
    if "nc" not in _cache:
        _cache["nc"] = _build_nc()
    return _cache["nc"]


def kernel(h: np.ndarray, A: np.ndarray) -> np.ndarray:
    from concourse.bass_utils import run_bass_kernel_spmd
    nc = _get_nc()
    h2 = np.ascontiguousarray(h.reshape(B * T, D).astype(np.float32))
    in_maps = [{"h": h2, "A": np.ascontiguousarray(A[i].astype(np.float32))}
               for i in range(NCORES)]
    res = run_bass_kernel_spmd(nc, in_maps, core_ids=list(range(NCORES)))
    outs = [res.results[i]["out"].reshape(B, T, D) for i in range(NCORES)]
    return np.concatenate(outs, axis=2)

